# revision 15
# baseline (speedup 1.0000x reference)
"""Trainium2 Bass kernel for nn_CosineProxy.

Reference computation (per task b):
    feats[n]  = blockmean_pool(x[b,n])            # (640,10,10) -> 800 dims
    proxy     = sum_n feats[n]                     # pooling is linear
    sim[n]    = <feats[n], proxy> / max(||feats[n]||*||proxy||, eps)
    out[b]    = sum_n sim[n] * x[b,n]

sim is scale-invariant, so block-SUM pooling is used instead of block-mean.
Sharding: pure data parallelism over B=256 tasks -> 32 tasks per core x 8 cores.

Per-core layout: x[b,n] (640*100 contiguous floats) lives in SBUF as
(128 partitions, 500 free) where partition p holds channels [5p,5p+5).
A 20-channel pooling block == 4 partitions x 5 in-partition channels.

Engine budget per 4-task group (HW-measured, vs 15.4us of DMA):
  DVE ~12.3us: 2x2 spatial pooling (s1/s2), FP/QP/QS Gram terms, the
    small cosine chain, and bf16 scaling of shots 1/3.
  ACT ~11.5us: bf16 scaling of shots 0/2/4, sqrt, and the PSUM->SBUF
    bf16 output copy.
  PE  ~13.0us: per-task pack matmuls (channel pooling across partitions),
    ones-block reductions, and 5 identity matmuls accumulating the
    pre-scaled shots in PSUM (LDWEIGHTS overlaps the matmul, so PE busy
    is the matmul slices only).
The output is written as bf16 (tolerance 2e-2; measured ~2e-3) which cuts
store traffic in half; the host upcasts to f32.

Software pipeline (iteration gi), tuned so no engine head-blocks on a
cross-engine latency chain:
  loads(gi) -> sqrt(gi-2) [ACT head: its input prod(gi-2) finished last
  iteration] -> FP/QP/QS(gi-1) [DVE, reads pk(gi-1) PSUM] -> mx/rs/
  simt(gi-2) -> rd(gi-1) [PE] -> rsb/prod(gi-1) -> per task t: scales
  (gi-2,t) on ACT+DVE, ea matmuls (gi-2,t), s1/s2(gi,t), pack(gi,t),
  output copy obc(gi-2,t) -> stores(gi-3).
Stores trail by a full stage so their sync-queue triggers never block
younger loads (no head-of-line blocking on the single HWDGE FIFO).

Tail: the last 4 tasks run as single-task groups (their own pack/rd/sims
on task slot 0) so the post-last-load dependency chain is one task's,
not four; their shot scales all run on DVE (tensor_scalar is ~4x faster
per op than ACT's activation copy and DVE is slack in the drain).
The consts load is emitted after group 0's x loads so the first x byte
is not delayed behind it.

Hard-won HW notes: GPSIMD tensor_scalar with per-partition scalar is
~7.6us/op on HW (CoreSim models 0.6us) and bulk GPSIMD streaming
contends with DVE's SBUF port - keep GPSIMD idle. Issuing stores via
nc.gpsimd.dma_start (SWDGE) or splitting load triggers onto the scalar
queue both regress by 20-40us; keep all DMA triggers on nc.sync.
DVE throughput is ~2 elem-widths/cycle across all streams, so access-
pattern contiguity tweaks (dh-first vs dw-first pooling) don't help;
only removing stream volume from DVE does. PSUM matmul start=True marks
the whole 2KB bank pending-zero: with task-outer accumulation only the
first shot-slice per bank may start and only the last may stop.
"""

import ml_dtypes
import numpy as np

import concourse.bacc as bacc
import concourse.mybir as mybir
import concourse.tile as tile
from concourse.bass_utils import run_bass_kernel_spmd

F32 = mybir.dt.float32
BF16 = mybir.dt.bfloat16
ADD = mybir.AluOpType.add
MULT = mybir.AluOpType.mult

P = 128          # SBUF partitions
N = 5            # shots
C = 640          # channels
HW = 100         # 10*10 spatial
CF = C // P      # 5 channels per partition
FREE = CF * HW   # 500 floats per partition per (b, n)
OS = 25          # pooled spatial size (5*5)
SF = CF * OS     # 125: spatially-pooled cols per (b, n)
EPS = 1e-8
NCORES = 8
B = 256
BC = B // NCORES  # 32 tasks per core
NSING = 0        # trailing single-task groups regress: FP/QP/QS cost is
                 # per-group (free-dim), so singles pay it 4x. Keep quads.


def consts_np() -> np.ndarray:
    """(128, 1152) bf16 consts: 4 packing mats, 4 ones-blocks, identity."""
    cs = np.zeros((P, 1152), np.float32)
    for t in range(4):
        for p in range(P):
            # B4t: route channel-partition p of task t to oc row t*32 + p//4
            cs[p, t * 128 + t * 32 + p // 4] = 1.0
        # OBt: ones on rows [32t, 32t+32), all 128 output columns
        cs[32 * t:32 * (t + 1), 512 + t * 128: 512 + (t + 1) * 128] = 1.0
    cs[np.arange(P), 1024 + np.arange(P)] = 1.0  # identity
    return cs.astype(ml_dtypes.bfloat16)


def build(bc: int = BC, reps: int = 1):
    """Build + compile the per-core Bass module for a bc-task shard."""
    assert bc % 4 == 0
    nc = bacc.Bacc("TRN2", target_bir_lowering=False, debug=False,
                   num_devices=NCORES)
    x_in = nc.dram_tensor("x", (bc, N, C, HW), F32, kind="ExternalInput")
    cs_in = nc.dram_tensor("consts", (P, 1152), BF16, kind="ExternalInput")
    out_d = nc.dram_tensor("out", (bc, C, HW), BF16, kind="ExternalOutput")

    xv = x_in[:].rearrange("b n (p cf) hw -> b p n (cf hw)", p=P, cf=CF)
    ov = out_d[:].rearrange("b (p cf) hw -> b p (cf hw)", p=P, cf=CF)

    # group schedule: quads then trailing singles
    nsing = NSING if bc > NSING else 0
    groups = [(b0, 4) for b0 in range(0, bc - nsing, 4)]
    groups += [(b0, 1) for b0 in range(bc - nsing, bc)]
    ng = len(groups)

    with tile.TileContext(nc) as tc:
        with (
            tc.tile_pool(name="cpool", bufs=1) as cpool,
            tc.tile_pool(name="xpool", bufs=14) as xpool,
            tc.tile_pool(name="wpool", bufs=3) as wpool,
            tc.tile_pool(name="s2pool", bufs=8) as s2pool,
            tc.tile_pool(name="spool", bufs=2) as spool,
            tc.tile_pool(name="pkpool", bufs=2, space="PSUM") as pkpool,
            tc.tile_pool(name="rdpool", bufs=1, space="PSUM") as rdpool,
            tc.tile_pool(name="eapool", bufs=3, space="PSUM") as eapool,
        ):
            csb = cpool.tile([P, 1152], BF16)
            lhs_pack = [csb[:, t * 128:(t + 1) * 128] for t in range(4)]
            lhs_ones = [csb[:, 512 + t * 128:512 + (t + 1) * 128]
                        for t in range(4)]
            lhs_eye = csb[:, 1024:1152]

            def load_dma(g):
                """Issue the task loads for group g."""
                b0, ts = groups[g]
                xts = []
                for t in range(ts):
                    xt = xpool.tile([P, N, FREE], F32, tag="x")
                    nc.sync.dma_start(xt[:], xv[b0 + t])
                    xts.append(xt)
                return xts

            def pool_pack_task(xt, t, ts, pkA, pkB):
                """Spatial 2x2 pooling (DVE) + pack matmuls (PE), one task.

                Emitting the pack matmuls per task lets PE overlap the next
                task's s1/s2 instead of waiting for all four."""
                s1 = wpool.tile([P, N * CF * 50], BF16, tag="s1")
                v = xt[:].rearrange(
                    "p n (ci h wo dw) -> p (n ci) h wo dw",
                    ci=CF, h=10, wo=5, dw=2)
                nc.vector.tensor_tensor(
                    out=s1[:].rearrange("p (a h wo) -> p a h wo",
                                        a=N * CF, wo=5),
                    in0=v[:, :, :, :, 0], in1=v[:, :, :, :, 1], op=ADD)
                s2 = s2pool.tile([P, N * SF], BF16, tag="s2")
                v1 = s1[:].rearrange("p (a ho dh wo) -> p a ho dh wo",
                                     a=N * CF, ho=5, dh=2)
                nc.vector.tensor_tensor(
                    out=s2[:].rearrange("p (a ho wo) -> p a ho wo",
                                        a=N * CF, wo=5),
                    in0=v1[:, :, :, 0, :], in1=v1[:, :, :, 1, :], op=ADD)
                for n in range(N):
                    pk = pkA[:, n * SF:(n + 1) * SF] if n < 3 else \
                        pkB[:, (n - 3) * SF:(n - 2) * SF]
                    # start marks the whole 2KB PSUM bank pending-zero, so
                    # only the first slice per bank starts and only the last
                    # slice per bank stops (pkA holds shots 0-2, pkB 3-4).
                    nc.tensor.matmul(pk, lhs_pack[t],
                                     s2[:, n * SF:(n + 1) * SF],
                                     start=(t == 0 and n in (0, 3)),
                                     stop=(t == ts - 1 and n in (2, 4)))

            def sims_a(g, pkA, pkB):
                """Pooled feats -> Gram terms -> rd reduction + prod (DVE+PE).

                Ends at prod = na^2 * nb^2 per (task, shot); the sqrt half
                of the chain runs next iteration so the ACT sqrt's input is
                always ready when ACT reaches it."""
                ts = groups[g][1]
                FP = spool.tile([P, 6 * OS], F32, tag="FP")
                nc.vector.tensor_reduce(
                    out=FP[:, 0:3 * OS],
                    in_=pkA[:].rearrange("p (j ci s) -> p j s ci", j=3, ci=CF),
                    axis=mybir.AxisListType.X, op=ADD)
                nc.vector.tensor_reduce(
                    out=FP[:, 3 * OS:5 * OS],
                    in_=pkB[:].rearrange("p (j ci s) -> p j s ci", j=2, ci=CF),
                    axis=mybir.AxisListType.X, op=ADD)
                nc.vector.tensor_reduce(
                    out=FP[:, 5 * OS:6 * OS],
                    in_=FP[:, 0:5 * OS].rearrange("p (n s) -> p s n", n=N),
                    axis=mybir.AxisListType.X, op=ADD)

                QP = spool.tile([P, 11 * OS], F32, tag="QP")
                nc.vector.tensor_tensor(
                    out=QP[:, 0:6 * OS].rearrange("p (b s) -> p b s", b=6),
                    in0=FP[:].rearrange("p (b s) -> p b s", b=6),
                    in1=FP[:, 5 * OS:6 * OS].rearrange(
                        "p (b s) -> p b s", b=1).broadcast_to((P, 6, OS)),
                    op=MULT)
                nc.vector.tensor_tensor(
                    out=QP[:, 6 * OS:11 * OS], in0=FP[:, 0:5 * OS],
                    in1=FP[:, 0:5 * OS], op=MULT)
                QS = spool.tile([P, 11], BF16, tag="QS")
                # bf16 Gram partials: ~0.2% rel err on norms, well within the
                # 2e-2 tolerance; lets the rd matmul run at 1 cycle/row.
                with nc.allow_low_precision(reason="bf16 Gram partials"):
                    nc.vector.tensor_reduce(
                        out=QS[:],
                        in_=QP[:].rearrange("p (q s) -> p q s", q=11),
                        axis=mybir.AxisListType.X, op=ADD)

                rd = rdpool.tile([P, 44], F32, tag="rd")
                for t in range(ts):
                    nc.tensor.matmul(rd[:, t * 11:(t + 1) * 11], lhs_ones[t],
                                     QS[:], start=True, stop=True)
                rsb = spool.tile([P, 44], F32, tag="rsb")
                nc.vector.tensor_copy(rsb[:, 0:11 * ts], rd[:, 0:11 * ts])
                rv = rsb[:, 0:11 * ts].rearrange("p (t q) -> p t q", t=ts)

                prod = spool.tile([P, 20], F32, tag="prod")
                nc.vector.tensor_tensor(
                    out=prod[:, 0:5 * ts].rearrange("p (t n) -> p t n", t=ts),
                    in0=rv[:, :, 6:11],
                    in1=rv[:, :, 5:6].broadcast_to((P, ts, 5)), op=MULT)
                return rsb, prod

            def sims_sqrt(g, prod):
                """ACT sqrt of prod(g) - emitted at ACT's queue head one
                iteration after prod so it never waits on DVE."""
                ts = groups[g][1]
                sq = spool.tile([P, 20], F32, tag="sq")
                nc.scalar.activation(sq[:, 0:5 * ts], prod[:, 0:5 * ts],
                                     mybir.ActivationFunctionType.Sqrt)
                return sq

            def sims_b(g, rsb, sq):
                """max/reciprocal/sim on DVE (sim = dot / max(na*nb, eps))."""
                ts = groups[g][1]
                rv = rsb[:, 0:11 * ts].rearrange("p (t q) -> p t q", t=ts)
                mx = spool.tile([P, 20], F32, tag="mx")
                nc.vector.tensor_scalar_max(mx[:, 0:5 * ts], sq[:, 0:5 * ts],
                                            EPS)
                rs = spool.tile([P, 20], F32, tag="rs")
                nc.vector.reciprocal(rs[:, 0:5 * ts], mx[:, 0:5 * ts])
                simt = spool.tile([P, 20], F32, tag="simt")
                nc.vector.tensor_tensor(
                    out=simt[:, 0:5 * ts].rearrange("p (t n) -> p t n", t=ts),
                    in0=rv[:, :, 0:5],
                    in1=rs[:, 0:5 * ts].rearrange("p (t n) -> p t n", t=ts),
                    op=MULT)
                return simt

            def weighted_task(xts, simt, t, dve_all):
                """Scale the 5 shots of task t (ACT: 0/2/4, DVE: 1/3; all on
                DVE for drain groups) -> 5 identity matmuls accumulate."""
                def st(n):
                    return simt[:, t * 5 + n:t * 5 + n + 1]
                avs = []
                for n in range(N):
                    av = wpool.tile([P, FREE], BF16, tag=f"a{n}")
                    if n in (1, 3) or dve_all:
                        nc.vector.tensor_scalar(
                            out=av[:], in0=xts[t][:, n, :], scalar1=st(n),
                            scalar2=None, op0=MULT)
                    else:
                        nc.scalar.activation(
                            av[:], xts[t][:, n, :],
                            mybir.ActivationFunctionType.Copy, scale=st(n))
                    avs.append(av)
                ea = eapool.tile([P, FREE], F32, tag="ea")
                for n in range(N):
                    nc.tensor.matmul(ea[:], lhs_eye, avs[n][:],
                                     start=(n == 0), stop=(n == N - 1))
                return ea

            def out_copy(ea):
                """ACT PSUM -> SBUF bf16 output copy."""
                ob = wpool.tile([P, FREE], BF16, tag="ob", bufs=8)
                nc.scalar.activation(ob[:], ea[:],
                                     mybir.ActivationFunctionType.Copy)
                return ob

            def weighted_stt(xts, simt, t):
                """All-DVE weighted sum for drain tasks: a tensor_scalar plus
                a chain of scalar_tensor_tensor ops, no ACT/PE involvement.
                ~2.7us of DVE per task - only worth it when DVE is otherwise
                idle (the last group's post-load drain) while ACT/PE chew
                through the other tasks in parallel."""
                def st(n):
                    return simt[:, t * 5 + n:t * 5 + n + 1]
                r = wpool.tile([P, FREE], F32, tag="r0", bufs=1)
                nc.vector.tensor_scalar(
                    out=r[:], in0=xts[t][:, 0, :], scalar1=st(0),
                    scalar2=None, op0=MULT)
                for n in (1, 2, 3):
                    rn = wpool.tile([P, FREE], F32, tag=f"r{n}", bufs=1)
                    nc.vector.scalar_tensor_tensor(
                        out=rn[:], in0=xts[t][:, n, :], scalar=st(n),
                        in1=r[:], op0=MULT, op1=ADD)
                    r = rn
                ob = wpool.tile([P, FREE], BF16, tag="ob", bufs=8)
                nc.vector.scalar_tensor_tensor(
                    out=ob[:], in0=xts[t][:, 4, :], scalar=st(4),
                    in1=r[:], op0=MULT, op1=ADD)
                return ob

            def stage_store(g, obs):
                """Deferred output DMA: ob(g) was produced one stage ago, so
                these triggers never block younger loads on the sync queue."""
                b0 = groups[g][0]
                for t, ob in enumerate(obs):
                    nc.sync.dma_start(ov[b0 + t], ob[:])

            total = reps * ng
            st_x, st_pk, st_sa, st_ob = {}, {}, {}, {}
            for gi in range(total + 3):
                if gi < total:
                    st_x[gi] = load_dma(gi % ng)
                if gi == 0:
                    # consts load trails group 0's x loads: the first x byte
                    # is not delayed behind it, and nothing needs it sooner.
                    nc.sync.dma_start(csb[:], cs_in[:])
                if 2 <= gi <= total + 1:
                    rsb, prod = st_sa[gi - 2]
                    sq = sims_sqrt((gi - 2) % ng, prod)
                if 1 <= gi <= total:
                    pkA, pkB = st_pk.pop(gi - 1)
                    st_sa[gi - 1] = sims_a((gi - 1) % ng, pkA, pkB)
                if 2 <= gi <= total + 1:
                    simt = sims_b((gi - 2) % ng, rsb, sq)
                    del st_sa[gi - 2]
                ld_now = gi < total
                wt_now = 2 <= gi <= total + 1
                if ld_now:
                    pkA = pkpool.tile([P, 3 * SF], F32, tag="pkA")
                    pkB = pkpool.tile([P, 2 * SF], F32, tag="pkB")
                    st_pk[gi] = (pkA, pkB)
                    ld_ts = groups[gi % ng][1]
                else:
                    ld_ts = 0
                if wt_now:
                    wt_g = (gi - 2) % ng
                    wt_ts = groups[wt_g][1]
                    dve_all = wt_ts == 1
                else:
                    wt_ts = 0
                if wt_now and gi - 2 == total - 1:
                    # Final group: no loads remain, so the weighted stage IS
                    # the drain tail. Split it across engines: tasks 2/3 on
                    # the ACT/PE path, tasks 0/1 as all-DVE stt chains in
                    # parallel -> the serial-PE ea tail halves.
                    eas = [(t, weighted_task(st_x[gi - 2], simt, t, False))
                           for t in (2, 3)]
                    obs = {t: weighted_stt(st_x[gi - 2], simt, t)
                           for t in (0, 1)}
                    for t, ea in eas:
                        obs[t] = out_copy(ea)
                    st_ob[gi - 2] = [obs[t] for t in range(4)]
                    del st_x[gi - 2]
                else:
                    eas = []
                    for t in range(4):
                        if t < wt_ts:
                            eas.append(weighted_task(st_x[gi - 2], simt, t,
                                                     dve_all))
                        if t < ld_ts:
                            pool_pack_task(st_x[gi][t], t, ld_ts, pkA, pkB)
                    if wt_now:
                        st_ob[gi - 2] = [out_copy(ea) for ea in eas]
                        del st_x[gi - 2]
                if gi >= 3:
                    stage_store((gi - 3) % ng, st_ob.pop(gi - 3))

    nc.compile()
    return nc


_CACHE = {}


def _get_nc(bc: int = BC):
    if bc not in _CACHE:
        _CACHE[bc] = build(bc)
    return _CACHE[bc]


def kernel(x: np.ndarray) -> np.ndarray:
    assert x.shape == (B, N, C, 10, 10) and x.dtype == np.float32
    nc = _get_nc(BC)
    cs = consts_np()
    shards = np.ascontiguousarray(x.reshape(NCORES, BC, N, C, HW))
    in_maps = [{"x": shards[i], "consts": cs} for i in range(NCORES)]
    res = run_bass_kernel_spmd(nc, in_maps, core_ids=list(range(NCORES)))
    out = np.concatenate([np.asarray(res.results[i]["out"])
                          for i in range(NCORES)])
    return out.reshape(B, C, 10, 10).astype(np.float32)
